# revision 11
# baseline (speedup 1.0000x reference)
"""GCNConv(128->128) + ReLU + Dropout(0.5) Trainium2 kernel, 8-core SPMD.

Math: with dinv = deg^-1/2 (self-loops included),
  out[v] = mask[v] * relu( (sum_{e:dst=v} dinv[src] x[src] + dinv[v] x[v]) dinv[v] @ W + b )
Row scaling commutes with @W, so features are aggregated in input space and
W is applied per 256-node output window afterwards.

Sharding: nodes + incident edges by destination across 8 cores; the scaled
feature table xs = dinv*x (bf16, split into 4 sub-tables so indices fit
int16) is replicated per core in DRAM. No collectives.

Per core pipeline (structure compile-time static, identical across cores):
  - nodes LPT-balanced into 256-node windows (per-core permutation,
    un-permuted on host)
  - per (node, src-quarter) slot lists padded to multiples of 4; per
    (window, quarter) runs padded to multiples of 128 slots
  - dma_gather (int16 idx, 256B rows, <=1024 rows/call, 4 SWDGE queues
    round-robin) pulls slot rows into SBUF
  - L1: static reduce-by-4 matmul (block one-hot at 4 PE column groups)
  - L2: dynamic one-hot segment-sum matmul, one-hot built on DVE via
    tensor_scalar(iota, is_equal dstv, mult dinv_dst)
  - @W matmul, relu+bias on ACT, dropout-mask multiply on DVE
  - output written feature-major [128, nodes]; host transposes/unpermutes
"""

import math
from contextlib import ExitStack

import numpy as np
import ml_dtypes

import concourse.bass as bass
import concourse.tile as tile
from concourse import bacc, mybir
from concourse.bass_utils import run_bass_kernel_spmd

# ---------------------------------------------------------------- constants
N = 100000
E = 1600000
D = 128
P = 0.5
N_CORES = 8
VN = N // N_CORES            # nodes per core
WIN = 256                    # output nodes per window
RED = 4                      # L1 static reduction factor
GW = 2                       # windows per gather group
MAX_IDX_PER_CALL = 1024
NQ = 4                       # SWDGE queues
SCRATCH = 32768
BF16 = mybir.dt.bfloat16
F32 = mybir.dt.float32
I16 = mybir.dt.int16


def _qd():
    return (N + 3) // 4


# ---------------------------------------------------------------- host prep
def _dropout_mask():
    import jax

    cpu = jax.devices("cpu")[0]
    with jax.default_device(cpu):
        keep = jax.random.bernoulli(jax.random.key(42), 1.0 - P, (N, D))
        return np.asarray(keep)


def _wrap16(stream):
    """[L] -> [128, L//16] per dma_gather idx convention (16-wrap, 8x repl)."""
    L = len(stream)
    assert L % 16 == 0
    w = np.empty((128, L // 16), np.int16)
    cols = stream.reshape(-1, 16).T.astype(np.int16)
    for a in range(8):
        w[16 * a : 16 * a + 16, :] = cols
    return w


def _lpt_windows(weights, nwin, cap):
    """Assign items to nwin bins (<=cap items each), minimizing max weight.
    Returns list of index-arrays (one per bin)."""
    order = np.argsort(weights)[::-1]
    loads = np.zeros(nwin)
    counts = np.zeros(nwin, np.int64)
    bins = [[] for _ in range(nwin)]
    for i in order:
        open_b = np.where(counts < cap)[0]
        b = open_b[np.argmin(loads[open_b])]
        bins[b].append(i)
        loads[b] += weights[i]
        counts[b] += 1
    return [np.array(sorted(b), np.int64) for b in bins]


def prep(x, edge_index, weight, bias):
    x = np.asarray(x, np.float32)
    src = np.asarray(edge_index[0], np.int64)
    dst = np.asarray(edge_index[1], np.int64)
    QD = _qd()
    TQ = QD + 1
    ZR = QD

    deg = np.bincount(dst, minlength=N).astype(np.float32) + 1.0
    dinv = (1.0 / np.sqrt(deg)).astype(np.float32)

    table = np.zeros((4 * TQ, D), ml_dtypes.bfloat16)
    xs = (x * dinv[:, None]).astype(ml_dtypes.bfloat16)
    for q in range(4):
        n_rows = min(QD, N - q * QD)
        if n_rows > 0:
            table[q * TQ : q * TQ + n_rows] = xs[q * QD : q * QD + n_rows]

    order = np.lexsort((src, dst))
    src_s = src[order]
    dst_s = dst[order]
    node_start = np.searchsorted(dst_s, np.arange(N + 1))

    NWIN = math.ceil(VN / WIN)

    eq = src_s // QD
    cnt = np.zeros((N, 4), np.int64)
    np.add.at(cnt, (dst_s, eq), 1)
    cnt[np.arange(N), np.arange(N) // QD] += 1
    pcnt = ((cnt + RED - 1) // RED) * RED

    # vectorized per-node quarter boundaries: position of first src >= q*QD
    # within each node's run = node_start[v] + cumsum over quarters of counts
    edge_cnt_vq = cnt.copy()
    edge_cnt_vq[np.arange(N), np.arange(N) // QD] -= 1  # remove self loop
    qstart = np.zeros((N, 5), np.int64)
    qstart[:, 1:] = np.cumsum(edge_cnt_vq, axis=1)
    qstart += node_start[:N, None]

    # LPT windows per core, weight = padded slots per node
    node_w = pcnt.sum(axis=1)
    perms = []      # per core: window-major node order [VN]
    for c in range(N_CORES):
        bins = _lpt_windows(node_w[c * VN : (c + 1) * VN], NWIN, WIN)
        perms.append(np.concatenate(bins) + c * VN)
    win_nodes = [
        [perms[c][w * WIN : min((w + 1) * WIN, VN)] for w in range(NWIN)]
        for c in range(N_CORES)
    ]

    n_c1 = np.zeros((NWIN, 4), np.int64)
    for c in range(N_CORES):
        for w in range(NWIN):
            s = pcnt[win_nodes[c][w]].sum(axis=0)
            n_c1[w] = np.maximum(n_c1[w], (s + 127) // 128)
    for w in range(NWIN):
        n_c1[w][3] += (-int(n_c1[w].sum())) % RED
    n_l2 = n_c1.sum(axis=1) // RED
    L_tot = int(n_l2.sum())
    K_tot = int(n_c1.sum())

    groups = [list(range(g, min(g + GW, NWIN))) for g in range(0, NWIN, GW)]

    mask = _dropout_mask()
    w_bf = np.asarray(weight, np.float32).astype(ml_dtypes.bfloat16)
    bias_f = np.asarray(bias, np.float32).reshape(D, 1)
    iota = np.broadcast_to(np.arange(WIN, dtype=np.float32), (128, WIN)).astype(
        ml_dtypes.bfloat16
    )
    m4 = np.zeros((128, 32), ml_dtypes.bfloat16)
    m4[np.arange(128), np.arange(128) // RED] = 1.0

    in_maps = []
    for c in range(N_CORES):
        idx_cols = [[] for _ in range(4)]
        dstv_cols = []
        dinv_cols = []
        for w in range(NWIN):
            nodes = win_nodes[c][w]
            dstv_w = []
            dinv_w = []
            for q in range(4):
                n_slots = int(n_c1[w][q]) * 128
                slots = np.full(n_slots, ZR, np.int64)
                cn = cnt[nodes, q]
                pc = pcnt[nodes, q]
                start = np.zeros(len(nodes), np.int64)
                start[1:] = np.cumsum(pc)[:-1]
                own = nodes // QD == q
                slots[start[own]] = nodes[own] - q * QD
                ecn = cn - own.astype(np.int64)
                tot = int(ecn.sum())
                if tot:
                    epos = np.repeat(start + own, ecn) + (
                        np.arange(tot) - np.repeat(np.cumsum(ecn) - ecn, ecn)
                    )
                    a = qstart[nodes, q]
                    evals = np.concatenate(
                        [src_s[ai : ai + n] for ai, n in zip(a, ecn)]
                    ) - q * QD
                    slots[epos] = evals
                idx_cols[q].append(slots)
                npart_q = n_slots // RED
                dq = np.full(npart_q, -1.0, np.float32)
                vq = np.zeros(npart_q, np.float32)
                real = int(pc.sum()) // RED
                p_of = np.repeat(np.arange(len(nodes)), pc // RED)
                dq[:real] = p_of.astype(np.float32)
                vq[:real] = dinv[nodes[p_of]]
                dstv_w.append(dq)
                dinv_w.append(vq)
            dstv_w = np.concatenate(dstv_w)
            dinv_w = np.concatenate(dinv_w)
            assert len(dstv_w) == int(n_l2[w]) * 128
            dstv_cols.append(dstv_w.reshape(-1, 128).T)
            dinv_cols.append(dinv_w.reshape(-1, 128).T)

        idx_wraps = [_wrap16(np.concatenate(idx_cols[q])) for q in range(4)]
        dstv_sb = np.concatenate(dstv_cols, axis=1)
        dinv_sb = np.concatenate(dinv_cols, axis=1)
        # mask in permuted (window-major) order
        maskT = (
            mask[perms[c], :].T.astype(np.float32) * 2.0
        ).astype(ml_dtypes.bfloat16)

        m = {
            "xs": table,
            "dstv": np.ascontiguousarray(dstv_sb),
            "dinvd": np.ascontiguousarray(dinv_sb),
            "maskt": np.ascontiguousarray(maskT),
            "wmat": w_bf,
            "biasv": bias_f,
            "m4": m4,
            "iota": iota,
        }
        for q in range(4):
            m[f"idx{q}"] = np.ascontiguousarray(idx_wraps[q])
        in_maps.append(m)

    meta = dict(
        n_c1=n_c1.tolist(),
        n_l2=[int(v) for v in n_l2],
        K_tot=K_tot,
        L_tot=L_tot,
        NWIN=NWIN,
        groups=groups,
        QD=QD,
        TQ=TQ,
        perms=[p.tolist() for p in perms],
    )
    return in_maps, meta


# ------------------------------------------------------------- bass builder
def build(meta, repeat=1):
    n_c1 = [list(map(int, r)) for r in meta["n_c1"]]
    n_l2 = meta["n_l2"]
    L_tot, NWIN = meta["L_tot"], meta["NWIN"]
    groups, TQ = meta["groups"], meta["TQ"]
    qlen = [sum(n_c1[w][q] for w in range(NWIN)) * 128 for q in range(4)]

    nc = bacc.Bacc(
        "TRN2",
        target_bir_lowering=False,
        debug=False,
        num_devices=N_CORES,
        num_swdge_queues=NQ,
        dynamic_dma_scratch_size=SCRATCH,
    )
    xs = nc.dram_tensor("xs", [4 * TQ, D], BF16, kind="ExternalInput").ap()
    idxd = [
        nc.dram_tensor(f"idx{q}", [128, qlen[q] // 16], I16, kind="ExternalInput").ap()
        for q in range(4)
    ]
    dstv = nc.dram_tensor("dstv", [128, L_tot], F32, kind="ExternalInput").ap()
    dinvd = nc.dram_tensor("dinvd", [128, L_tot], F32, kind="ExternalInput").ap()
    maskt = nc.dram_tensor("maskt", [128, VN], BF16, kind="ExternalInput").ap()
    wmat = nc.dram_tensor("wmat", [D, D], BF16, kind="ExternalInput").ap()
    biasv = nc.dram_tensor("biasv", [D, 1], F32, kind="ExternalInput").ap()
    m4 = nc.dram_tensor("m4", [128, 32], BF16, kind="ExternalInput").ap()
    iota = nc.dram_tensor("iota", [128, WIN], BF16, kind="ExternalInput").ap()
    outT = nc.dram_tensor("outT", [128, VN], F32, kind="ExternalOutput").ap()

    grp_q_chunks = [
        [sum(n_c1[w][q] for w in grp) for q in range(4)] for grp in groups
    ]
    max_grp = max(sum(gq) for gq in grp_q_chunks)

    with tile.TileContext(nc) as tc, ExitStack() as ctx:
        statics = ctx.enter_context(tc.tile_pool(name="statics", bufs=1))
        gpool = ctx.enter_context(tc.tile_pool(name="g", bufs=2))
        ppool = ctx.enter_context(tc.tile_pool(name="part", bufs=6))
        m2pool = ctx.enter_context(tc.tile_pool(name="m2", bufs=6))
        apool = ctx.enter_context(tc.tile_pool(name="agg", bufs=2))
        opool = ctx.enter_context(tc.tile_pool(name="osb", bufs=3))
        ps1 = ctx.enter_context(tc.tile_pool(name="ps1", bufs=4, space="PSUM"))
        ps2 = ctx.enter_context(tc.tile_pool(name="ps2", bufs=2, space="PSUM"))
        ps3 = ctx.enter_context(tc.tile_pool(name="ps3", bufs=2, space="PSUM"))

        def load_static(name, ap, shape, dtype):
            t = statics.tile(shape, dtype, tag=name)
            nc.sync.dma_start(out=t[:], in_=ap)
            return t

        m4_sb = load_static("m4", m4, [128, 32], BF16)
        iota_sb = load_static("iota", iota, [128, WIN], BF16)
        w_sb = load_static("wmat", wmat, [D, D], BF16)
        bias_sb = load_static("biasv", biasv, [D, 1], F32)
        dstv_sb = load_static("dstv", dstv, [128, L_tot], F32)
        dinvd_sb = load_static("dinvd", dinvd, [128, L_tot], F32)
        mask_sb = load_static("maskt", maskt, [128, VN], BF16)
        idx_sb = [
            load_static(f"idx{q}", idxd[q], [128, qlen[q] // 16], I16)
            for q in range(4)
        ]

        call_seq = 0
        for _rep in range(repeat):
            qpos = [0, 0, 0, 0]
            l0 = 0
            for gi, grp in enumerate(groups):
                g = gpool.tile([128, max_grp, D], BF16, tag="g")
                col = 0
                gbase = {}
                # build the group's call list, then issue round-robin so
                # Tile's 8 DMASW lanes map consistently onto the 4 queues
                calls = []
                for q in range(4):
                    nch = grp_q_chunks[gi][q]
                    cc = 0
                    for w in grp:
                        gbase[(w, q)] = col + cc
                        cc += n_c1[w][q]
                    s = 0
                    while s < nch * 128:
                        n = min(MAX_IDX_PER_CALL, nch * 128 - s)
                        calls.append((q, col + s // 128, qpos[q] + s, n))
                        s += n
                    qpos[q] += nch * 128
                    col += nch
                for (q, gcol, ip, n) in calls:
                    nc.gpsimd.dma_gather(
                        out_ap=g[:, gcol : gcol + n // 128, :],
                        in_ap=xs[q * TQ : (q + 1) * TQ, :],
                        idxs_ap=idx_sb[q][:, ip // 16 : (ip + n) // 16],
                        num_idxs=n,
                        num_idxs_reg=n,
                        elem_size=D,
                        queue_num=call_seq % NQ,
                    )
                    call_seq += 1

                for w in grp:
                    nl2 = n_l2[w]
                    wn = min(WIN, VN - WIN * w)
                    chunks = []
                    for q in range(4):
                        chunks.extend(gbase[(w, q)] + j for j in range(n_c1[w][q]))
                    assert len(chunks) == RED * nl2

                    agg_ps = ps2.tile([128, WIN], F32)
                    for t in range(nl2):
                        p1 = ps1.tile([128, 128], F32)
                        for cq in range(RED):
                            nc.tensor.matmul(
                                p1[32 * cq : 32 * cq + 32, :],
                                lhsT=m4_sb[:],
                                rhs=g[:, chunks[RED * t + cq], :],
                                start=True,
                                stop=True,
                                tile_position=(0, 32 * cq),
                            )
                        part_sb = ppool.tile([128, 128], BF16, tag="part")
                        if t % 2 == 0:
                            nc.vector.tensor_copy(part_sb[:], p1[:])
                        else:
                            nc.scalar.activation(
                                part_sb[:], p1[:], mybir.ActivationFunctionType.Copy
                            )
                        m2 = m2pool.tile([128, WIN], BF16, tag="m2")
                        nc.vector.tensor_scalar(
                            out=m2[:],
                            in0=iota_sb[:],
                            scalar1=dstv_sb[:, l0 + t : l0 + t + 1],
                            scalar2=dinvd_sb[:, l0 + t : l0 + t + 1],
                            op0=mybir.AluOpType.is_equal,
                            op1=mybir.AluOpType.mult,
                        )
                        nc.tensor.matmul(
                            agg_ps[:],
                            lhsT=part_sb[:],
                            rhs=m2[:],
                            start=(t == 0),
                            stop=(t == nl2 - 1),
                        )

                    agg_sb = apool.tile([128, WIN], BF16, tag="agg")
                    nc.scalar.activation(
                        agg_sb[:], agg_ps[:], mybir.ActivationFunctionType.Copy
                    )
                    out_ps = ps3.tile([128, WIN], F32)
                    nc.tensor.matmul(
                        out_ps[:], lhsT=w_sb[:], rhs=agg_sb[:], start=True, stop=True
                    )
                    out_sb = opool.tile([128, WIN], F32, tag="osb")
                    nc.scalar.activation(
                        out_sb[:],
                        out_ps[:],
                        mybir.ActivationFunctionType.Relu,
                        bias=bias_sb[:, 0:1],
                    )
                    res_sb = opool.tile([128, WIN], F32, tag="res")
                    nc.vector.tensor_tensor(
                        out=res_sb[:, :wn],
                        in0=out_sb[:, :wn],
                        in1=mask_sb[:, WIN * w : WIN * w + wn],
                        op=mybir.AluOpType.mult,
                    )
                    nc.sync.dma_start(
                        out=outT[:, WIN * w : WIN * w + wn], in_=res_sb[:, :wn]
                    )
                    l0 += nl2

    nc.finalize()
    return nc


def assemble(meta, results):
    """[core results] -> full [N, D] output (transpose + un-permute)."""
    out = np.empty((N, D), np.float32)
    for c in range(N_CORES):
        perm = np.asarray(meta["perms"][c], np.int64)
        out[perm] = results[c]["outT"].T
    return out


# ---------------------------------------------------------------- kernel()
def kernel(**inputs):
    in_maps, meta = prep(
        inputs["x"], inputs["edge_index"], inputs["weight"], inputs["bias"]
    )
    nc = build(meta)
    res = run_bass_kernel_spmd(nc, in_maps, core_ids=list(range(N_CORES)))
    return assemble(meta, res.results)
